# revision 32
# baseline (speedup 1.0000x reference)
"""Multi-head attention + output dense + LayerNorm + residual, on 8 NeuronCores.

Sharding: core c -> (batch b = c//2, query-half hf = c%2). Each core runs the
full 16-head attention for its 1024 queries against its batch's full 2048
keys (K/V projections are recomputed per query-half; no collectives needed).
The host reorders tokens so each core's queries are always rows 0:1024 of its
input slab -- key order is softmax-invariant as long as the mask is permuted
identically, so the device program is completely SPMD-uniform.

Bass kernel layout (v6):
  - The additive mask folds into V instead of the scores: with w = exp(m/8),
    softmax((s+m)/8) @ V == (exp(s/8) @ (V*w)) / (exp(s/8) @ w). V's 65th
    column holds w, so row 64 of the ctx accumulation is the denominator.
    Scores then have a pure 64-deep contraction (no mask row), so the two
    heads of a pair run CONCURRENTLY as 64x128 PE row tiles (T0: partitions
    0-63 = head A dims, T8: 64-127 = head B) into two PSUM banks -- 2x score
    throughput vs the K=65 single-tile form.
  - exp is one ScalarE activation per key chunk over both heads' score banks
    ([128, 2, 512] -> bf16 SBUF). All 16 es tiles of a query block stay
    resident, so both heads' ctx accumulations (full-128 contraction, M=65)
    run as one dense 128-mode stretch -- only 2 PE tiling-mode switches per
    query block.
  - Q/K projections are baseline-style full-mode matmuls; bias lands in the
    per-partition scalar of the evacuation (tensor_scalar_add at full 128
    width for both heads at once). V's evacuation fuses the w mask scale as a
    tensor_scalar_mul; V/out-dense biases ride K=1 ones-row matmuls.
  - 1/denominator (reciprocal_approx_fast) is broadcast across partitions by
    a DRAM-bounce DMA with a 0-stride AP (no PE broadcast matmuls), then one
    [128,512] DVE multiply normalizes both heads' ctx.
  - Output projection consumes normalized ctx^T directly; LayerNorm runs on
    DVE (bn_stats/bn_aggr), residual+beta adds on GpSimd.
"""

import numpy as np

B, S, H, NH = 4, 2048, 1024, 16
HD = H // NH  # 64
SQ = S // 2  # queries per core
NCORES = 8
NPAIR = NH // 2  # head pairs
NCI = H // 128  # 8 contraction chunks
NKC = S // 128  # 16 key chunks
EPS = 1e-12

_cache = {}


def _build():
    import concourse.bass as bass
    import concourse.bacc as bacc
    import concourse.mybir as mybir
    import concourse.tile as tile

    fp32 = mybir.dt.float32
    bf16 = mybir.dt.bfloat16
    AF = mybir.ActivationFunctionType
    OP = mybir.AluOpType

    nc = bacc.Bacc("TRN2", target_bir_lowering=False, debug=False)

    xkv = nc.dram_tensor("xkv", [S, H], fp32, kind="ExternalInput").ap()
    xtb_d = nc.dram_tensor("xtb", [NKC, 128, NCI, 128], bf16, kind="ExternalInput").ap()
    mask8_d = nc.dram_tensor("mask8", [S], fp32, kind="ExternalInput").ap()
    wq_d = nc.dram_tensor("wq", [H, H], bf16, kind="ExternalInput").ap()
    wk_d = nc.dram_tensor("wk", [H, H], bf16, kind="ExternalInput").ap()
    wv_d = nc.dram_tensor("wv", [H, H], bf16, kind="ExternalInput").ap()
    wd_d = nc.dram_tensor("wd", [H, H], bf16, kind="ExternalInput").ap()
    bq_d = nc.dram_tensor("bq", [H], fp32, kind="ExternalInput").ap()
    bk_d = nc.dram_tensor("bk", [H], fp32, kind="ExternalInput").ap()
    bv_d = nc.dram_tensor("bv", [H], fp32, kind="ExternalInput").ap()
    bd_d = nc.dram_tensor("bd", [H], fp32, kind="ExternalInput").ap()
    gamma_d = nc.dram_tensor("gamma", [H], fp32, kind="ExternalInput").ap()
    beta_d = nc.dram_tensor("beta", [H], fp32, kind="ExternalInput").ap()
    out_d = nc.dram_tensor("out", [SQ, H], fp32, kind="ExternalOutput").ap()

    def bcast(ap, n):
        return bass.AP(tensor=ap.tensor, offset=ap.offset, ap=[[0, n]] + list(ap.ap))

    with tile.TileContext(nc) as tc:
        with (
            tc.tile_pool(name="consts", bufs=1) as consts,
            tc.tile_pool(name="ctxT", bufs=1) as ctxt_pool,
            tc.tile_pool(name="dscr2", bufs=1, space="DRAM") as dram_pool2,
        ):
            # --- constants ---
            bqT = consts.tile([128, NCI], fp32)
            nc.sync.dma_start(out=bqT, in_=bq_d.rearrange("(c p) -> p c", p=128))
            bkT = consts.tile([128, NCI], fp32)
            nc.sync.dma_start(out=bkT, in_=bk_d.rearrange("(c p) -> p c", p=128))
            eps_sb = consts.tile([128, 1], fp32)
            nc.vector.memset(eps_sb, EPS)
            bvT = consts.tile([128, NCI], bf16, name="bvT")
            nc.gpsimd.dma_start(out=bvT, in_=bv_d.rearrange("(c p) -> p c", p=128))
            bd_row = consts.tile([1, H], fp32)
            nc.gpsimd.dma_start(out=bd_row, in_=bcast(bd_d, 1))
            # w = exp(mask/8) per key token, [128, NKC]
            m8_sb = consts.tile([128, NKC], fp32, name="m8_sb")
            nc.sync.dma_start(out=m8_sb, in_=mask8_d.rearrange("(c p) -> p c", p=128))
            w_sb = consts.tile([128, NKC], fp32, name="w_sb")
            nc.scalar.activation(out=w_sb, in_=m8_sb, func=AF.Exp, scale=1.0 / 64.0)
            wb_sb = consts.tile([128, NKC], bf16, name="wb_sb")
            nc.vector.tensor_copy(out=wb_sb, in_=w_sb)
            wpad = consts.tile([128, NKC, 64], bf16, name="wpad")
            nc.vector.memset(wpad, 0.0)
            nc.vector.tensor_copy(out=wpad[:, :, 0:1], in_=wb_sb.unsqueeze(2))

            # ctxT[hl*64+d, hp, q] = ctx[q, (hp*2+hl)*64+d] / sumexp
            ctxt = ctxt_pool.tile([128, NPAIR, SQ], bf16)

            wd_sb = ctxt_pool.tile([128, NCI, H], bf16, name="wd_sb")

            ctx_mid = tc.tile_pool(name="midA", bufs=1)
            midA = ctx_mid.__enter__()
            xt = midA.tile([128, NKC, NCI, 128], bf16, name="xt")
            wq_full = midA.tile([128, NCI, H], bf16, name="wq_full")
            wk_full = midA.tile([128, NCI, H], bf16, name="wk_full")
            v_all = midA.tile([128, NKC, NH, 65], bf16, name="v_all")
            # w column (65th) for every head: v_all[t, kc, h, 64] = w[t, kc]
            for h in range(NH):
                nc.vector.tensor_copy(out=v_all[:, :, h, 64:65], in_=wb_sb.unsqueeze(2))

            # --- phase 1: V for all heads: v_all[t, kc, h, 0:64] = (xWv+bv)*w ---
            with (
                tc.tile_pool(name="wvf", bufs=1) as wvf_pool,
                tc.tile_pool(name="vp", bufs=2, space="PSUM") as vp,
            ):
                wv_full = wvf_pool.tile([128, NCI, H], bf16, name="wv_full")
                wv_re = wv_d.rearrange("(c p) n -> p c n", p=128)
                nc.sync.dma_start(out=wv_full[:, :, 0:512], in_=wv_re[:, :, 0:512])
                nc.sync.dma_start(out=xt[:, 0], in_=xtb_d[0])
                nc.sync.dma_start(out=wv_full[:, :, 512:H], in_=wv_re[:, :, 512:H])
                for tch in range(1, NKC):
                    nc.sync.dma_start(out=xt[:, tch], in_=xtb_d[tch])
                nc.sync.dma_start(
                    out=wq_full, in_=wq_d.rearrange("(c p) n -> p c n", p=128)
                )
                nc.sync.dma_start(
                    out=wk_full, in_=wk_d.rearrange("(c p) n -> p c n", p=128)
                )
                for tb in range(NKC):
                    for nh in range(2):
                        nsl = slice(nh * 512, (nh + 1) * 512)
                        pv = vp.tile([128, 512], fp32)
                        for ci in range(NCI):
                            nc.tensor.matmul(
                                pv,
                                xt[:, tb, ci, :],
                                wv_full[:, ci, nsl],
                                start=(ci == 0),
                                stop=(ci == NCI - 1),
                            )
                        nc.vector.tensor_scalar_mul(
                            out=v_all[:, tb, nh * 8 : (nh + 1) * 8, 0:64],
                            in0=pv.rearrange("p (a b) -> p a b", a=8),
                            scalar1=w_sb[:, tb : tb + 1],
                        )

            # prefetch Wd during attention (consumed by phase 3)
            nc.sync.dma_start(
                out=wd_sb, in_=wd_d.rearrange("(c p) n -> p c n", p=128)
            )

            # --- phase 2: per head-pair projections + attention ---
            with (
                tc.tile_pool(name="qtpool", bufs=2) as qt_pool,
                tc.tile_pool(name="ktpool", bufs=2) as kt_pool,
                tc.tile_pool(name="espool", bufs=20) as es_pool,
                tc.tile_pool(name="sums", bufs=2) as sums_pool,
                tc.tile_pool(name="pbpool", bufs=2) as pb_pool,
                tc.tile_pool(name="dscr", bufs=2, space="DRAM") as dram_pool,
                tc.tile_pool(name="projp", bufs=2, space="PSUM") as projp,
                tc.tile_pool(name="sp", bufs=2, space="PSUM") as sp,
                tc.tile_pool(name="cp", bufs=1, space="PSUM") as cp,
            ):
                # deferred ctx work: thunks from the previous query block get
                # woven into the next block's ACT-paced scores loop so the PE
                # never idles waiting for exp results
                from collections import deque

                pending = deque()

                def drain(n):
                    for _ in range(n):
                        if not pending:
                            return
                        pending.popleft()()

                def make_ctx_thunks(hp, qsl, es_tiles, last=False):
                    state = {}

                    def mm(h, kc):
                        # col tile h: ctx for head h into partition half h,
                        # then the w-row denominator into the second bank
                        if "pc" not in state:
                            state["pc"] = cp.tile([128, 2, 512], fp32, tag="pc", name="pc")
                        psl = slice(h * 64, (h + 1) * 64)
                        nc.tensor.matmul(
                            state["pc"][psl, 0, :],
                            v_all[:, kc, hp * 2 + h, 0:64],
                            es_tiles[kc][:, h, :],
                            start=(kc == 0),
                            stop=(kc == NKC - 1),
                            skip_group_check=True,
                        )
                        nc.tensor.matmul(
                            state["pc"][psl, 1, :],
                            wpad[:, kc, :],
                            es_tiles[kc][:, h, :],
                            start=(kc == 0),
                            stop=(kc == NKC - 1),
                            skip_group_check=True,
                        )

                    def evac():
                        pc = state["pc"]
                        denA = sums_pool.tile([1, 512], fp32, tag="denA", name="denA")
                        denB = sums_pool.tile([1, 512], fp32, tag="denB", name="denB")
                        recA = sums_pool.tile([1, 512], fp32, tag="recA", name="recA")
                        recB = sums_pool.tile([1, 512], fp32, tag="recB", name="recB")
                        nc.vector.tensor_copy(
                            out=ctxt[:, hp, qsl], in_=pc[:, 0, :]
                        )
                        nc.vector.tensor_copy(out=denA, in_=pc[0:1, 1, :])
                        nc.vector.reciprocal_approx_fast(out=recA, in_=denA)
                        nc.vector.tensor_copy(out=denB, in_=pc[64:65, 1, :])
                        nc.vector.reciprocal_approx_fast(out=recB, in_=denB)
                        pb = pb_pool.tile([128, 512], fp32, name="pb")
                        scrA = dram_pool.tile([1, 512], fp32, tag="scrA", name="scrA")
                        nc.sync.dma_start(out=scrA, in_=recA)
                        nc.sync.dma_start(
                            out=pb[0:64, :], in_=bcast(scrA.squeeze(0), 64)
                        )
                        scrB = dram_pool.tile([1, 512], fp32, tag="scrB", name="scrB")
                        nc.sync.dma_start(out=scrB, in_=recB)
                        nc.sync.dma_start(
                            out=pb[64:128, :], in_=bcast(scrB.squeeze(0), 64)
                        )
                        nc.vector.tensor_tensor(
                            out=ctxt[:, hp, qsl],
                            in0=ctxt[:, hp, qsl],
                            in1=pb,
                            op=OP.mult,
                        )

                    thunks = []
                    for kc in range(NKC):
                        for h in range(2):
                            thunks.append(lambda h=h, kc=kc: mm(h, kc))
                    thunks.append(evac)
                    return thunks

                # projections as thunks too: queued two head-pairs ahead so
                # the ACT-paced scores loops always have PE fill work
                proj_state = {}

                def make_proj_thunks(hp):
                    cols = slice(hp * 128, (hp + 1) * 128)
                    st = {}
                    proj_state[hp] = st
                    thunks = []

                    def k_mm(tb, ci):
                        if "ktp" not in st:
                            st["ktp"] = kt_pool.tile([128, S], bf16, name="ktp")
                        if ("pk", tb) not in st:
                            st[("pk", tb)] = projp.tile(
                                [128, 512], fp32, tag="proj", name="pk"
                            )
                        nc.tensor.matmul(
                            st[("pk", tb)],
                            wk_full[:, ci, cols],
                            xt[:, tb * 4 : (tb + 1) * 4, ci, :],
                            start=(ci == 0),
                            stop=(ci == NCI - 1),
                        )

                    def k_evac(tb):
                        nc.vector.tensor_scalar_add(
                            out=st["ktp"][:, tb * 512 : (tb + 1) * 512],
                            in0=st[("pk", tb)],
                            scalar1=bkT[:, hp : hp + 1],
                        )

                    def q_mm(qb, ci):
                        if "qtp" not in st:
                            st["qtp"] = qt_pool.tile([128, SQ], bf16, name="qtp")
                        if ("pq", qb) not in st:
                            st[("pq", qb)] = projp.tile(
                                [128, 512], fp32, tag="proj", name="pq"
                            )
                        nc.tensor.matmul(
                            st[("pq", qb)],
                            wq_full[:, ci, cols],
                            xt[:, qb * 4 : (qb + 1) * 4, ci, :],
                            start=(ci == 0),
                            stop=(ci == NCI - 1),
                        )

                    def q_evac(qb):
                        nc.vector.tensor_scalar_add(
                            out=st["qtp"][:, qb * 512 : (qb + 1) * 512],
                            in0=st[("pq", qb)],
                            scalar1=bqT[:, hp : hp + 1],
                        )

                    for tb in range(S // 512):
                        for ci in range(NCI):
                            thunks.append(lambda tb=tb, ci=ci: k_mm(tb, ci))
                        thunks.append(lambda tb=tb: k_evac(tb))
                    for qb in range(SQ // 512):
                        for ci in range(NCI):
                            thunks.append(lambda qb=qb, ci=ci: q_mm(qb, ci))
                        thunks.append(lambda qb=qb: q_evac(qb))

                    def done():
                        st["complete"] = True

                    thunks.append(done)
                    return thunks

                def queue_proj(hp):
                    if hp < NPAIR:
                        pending.extend(make_proj_thunks(hp))

                queue_proj(0)
                drain(len(pending))  # head-pair 0 projections run dense
                queue_proj(1)

                for hp in range(NPAIR):
                    # make sure this pair's projections are fully emitted
                    while not proj_state[hp].get("complete"):
                        drain(1)
                    ktp = proj_state[hp]["ktp"]
                    qtp = proj_state[hp]["qtp"]

                    # attention: scores paced by ACT; pending ctx/proj thunks
                    # fill the PE gaps between score pairs
                    for qb in range(SQ // 512):
                        qsl = slice(qb * 512, (qb + 1) * 512)
                        es_tiles = []
                        for kc2 in range(0, NKC, 2):
                            for kc in (kc2, kc2 + 1):
                                ksl = slice(kc * 128, (kc + 1) * 128)
                                ps = sp.tile([128, 2, 512], fp32, tag="ps")
                                nc.tensor.matmul(
                                    ps[:, 0, :], ktp[0:64, ksl], qtp[0:64, qsl],
                                    start=True, stop=True,
                                )
                                nc.tensor.matmul(
                                    ps[:, 1, :], ktp[64:128, ksl], qtp[64:128, qsl],
                                    start=True, stop=True,
                                )
                                es = es_pool.tile([128, 2, 512], bf16, name="es")
                                nc.scalar.activation(
                                    out=es.rearrange("p a b -> p (a b)"),
                                    in_=ps.rearrange("p a b -> p (a b)"),
                                    func=AF.Exp,
                                    scale=0.125,
                                )
                                es_tiles.append(es)
                            drain(12)

                        pending.extend(
                            make_ctx_thunks(
                                hp, qsl, es_tiles,
                                last=(hp == NPAIR - 1 and qb == SQ // 512 - 1),
                            )
                        )
                    queue_proj(hp + 2)
                drain(len(pending))

            ctx_mid.__exit__(None, None, None)

            # --- phase 3: output projection + LayerNorm + residual ---
            with (
                tc.tile_pool(name="hid", bufs=4) as hid_pool,
                tc.tile_pool(name="lnbuf", bufs=4) as lnbuf,
                tc.tile_pool(name="op", bufs=3, space="PSUM") as op_pool,
                tc.tile_pool(name="opbd", bufs=1, space="PSUM") as opbd_pool,
            ):
                # total output bias bdt = bd + bv @ Wd, broadcast to 128 rows
                # (the V bias is folded here: (ctx/den + bv)@Wd + bd)
                gamma_b = hid_pool.tile([128, H], fp32, name="gamma_b")
                nc.sync.dma_start(out=gamma_b, in_=bcast(gamma_d, 128))
                beta_b = hid_pool.tile([128, H], fp32, name="beta_b")
                nc.sync.dma_start(out=beta_b, in_=bcast(beta_d, 128))
                bdt_b = hid_pool.tile([128, H], fp32, name="bdt_b")
                scr_bdt = dram_pool2.tile([1, H], fp32, name="scr_bdt")
                pbd = opbd_pool.tile([1, 1024], fp32, tag="pbd", name="pbd")
                bdt = lnbuf.tile([1, H], fp32, tag="bdt", name="bdt")
                for nb in range(2):
                    nsl = slice(nb * 512, (nb + 1) * 512)
                    for ci in range(NCI):
                        nc.tensor.matmul(
                            pbd[:, nsl],
                            bvT[:, ci : ci + 1],
                            wd_sb[:, ci, nsl],
                            start=(ci == 0),
                            stop=(ci == NCI - 1),
                        )
                nc.vector.tensor_tensor(
                    out=bdt, in0=pbd, in1=bd_row, op=OP.add
                )
                nc.sync.dma_start(out=scr_bdt, in_=bdt)
                nc.sync.dma_start(out=bdt_b, in_=bcast(scr_bdt.squeeze(0), 128))
                for qt in range(SQ // 128):
                    qsl = slice(qt * 128, (qt + 1) * 128)
                    hid = hid_pool.tile([128, H], fp32)
                    for nb in range(2):
                        nsl = slice(nb * 512, (nb + 1) * 512)
                        po = op_pool.tile([128, 512], fp32)
                        for ci in range(NCI):
                            nc.tensor.matmul(
                                po,
                                ctxt[:, ci, qsl],
                                wd_sb[:, ci, nsl],
                                start=(ci == 0),
                                stop=(ci == NCI - 1),
                            )
                        nc.vector.tensor_tensor(
                            out=hid[:, nsl], in0=po, in1=bdt_b[:, nsl], op=OP.add
                        )
                    # LayerNorm stats
                    stats = lnbuf.tile([128, 2, 6], fp32, tag="stats")
                    for sg in range(2):
                        nc.vector.bn_stats(
                            out=stats[:, sg, :], in_=hid[:, sg * 512 : (sg + 1) * 512]
                        )
                    mv = lnbuf.tile([128, 2], fp32, tag="mv")
                    nc.vector.bn_aggr(out=mv, in_=stats)
                    rstd = lnbuf.tile([128, 1], fp32, tag="rstd")
                    nc.scalar.activation(
                        out=rstd, in_=mv[:, 1:2], func=AF.Sqrt, bias=eps_sb
                    )
                    nc.vector.reciprocal(rstd, rstd)
                    # residual + beta (overlaps with stats)
                    x_res = lnbuf.tile([128, H], fp32, tag="xres")
                    nc.sync.dma_start(out=x_res, in_=xkv[qsl, :])
                    xbeta = lnbuf.tile([128, H], fp32, tag="xbeta")
                    nc.gpsimd.tensor_tensor(out=xbeta, in0=x_res, in1=beta_b, op=OP.add)
                    # (hid - mu) * rstd * gamma + (x + beta)
                    norm = lnbuf.tile([128, H], fp32, tag="norm")
                    nc.vector.tensor_scalar(
                        out=norm,
                        in0=hid,
                        scalar1=mv[:, 0:1],
                        scalar2=rstd,
                        op0=OP.subtract,
                        op1=OP.mult,
                    )
                    nc.vector.tensor_mul(norm, norm, gamma_b)
                    final = lnbuf.tile([128, H], fp32, tag="final")
                    nc.vector.tensor_tensor(out=final, in0=norm, in1=xbeta, op=OP.add)
                    nc.sync.dma_start(out=out_d[qsl, :], in_=final)

    nc.compile()
    return nc


def get_nc():
    if "nc" not in _cache:
        _cache["nc"] = _build()
    return _cache["nc"]


def make_in_maps(inputs):
    q = np.ascontiguousarray(np.asarray(inputs["query"], dtype=np.float32))
    am = np.asarray(inputs["attention_mask"], dtype=np.float32).reshape(B, S)
    import ml_dtypes

    bfl = ml_dtypes.bfloat16
    shared = {
        "wq": np.ascontiguousarray(np.asarray(inputs["Wq"], np.float32).astype(bfl)),
        "wk": np.ascontiguousarray(np.asarray(inputs["Wk"], np.float32).astype(bfl)),
        "wv": np.ascontiguousarray(np.asarray(inputs["Wv"], np.float32).astype(bfl)),
        "wd": np.ascontiguousarray(np.asarray(inputs["Wd"], np.float32).astype(bfl)),
        "bq": np.asarray(inputs["bq"], np.float32),
        "bk": np.asarray(inputs["bk"], np.float32),
        "bv": np.asarray(inputs["bv"], np.float32),
        "bd": np.asarray(inputs["bd"], np.float32),
        "gamma": np.asarray(inputs["ln_gamma"], np.float32),
        "beta": np.asarray(inputs["ln_beta"], np.float32),
    }
    in_maps = []
    for c in range(NCORES):
        b, hf = c // 2, c % 2
        # queries first, then the other half -- key order is softmax-invariant
        if hf == 0:
            xkv = q[b]
            mask = am[b]
        else:
            xkv = np.concatenate([q[b, SQ:], q[b, :SQ]], axis=0)
            mask = np.concatenate([am[b, SQ:], am[b, :SQ]], axis=0)
        m = dict(shared)
        m["xkv"] = np.ascontiguousarray(xkv)
        xtc = xkv.reshape(S // 128, 128, H // 128, 128).transpose(0, 3, 2, 1)
        m["xtb"] = np.ascontiguousarray(xtc.astype(bfl))
        m["mask8"] = np.ascontiguousarray(mask * 8.0)
        in_maps.append(m)
    return in_maps


def assemble(results):
    out = np.empty((B, S, H), dtype=np.float32)
    for c in range(NCORES):
        b, hf = c // 2, c % 2
        out[b, hf * SQ : (hf + 1) * SQ, :] = results[c]["out"]
    return out


def kernel(**inputs):
    from concourse.bass_utils import run_bass_kernel_spmd

    nc = get_nc()
    in_maps = make_in_maps(inputs)
    res = run_bass_kernel_spmd(nc, in_maps, core_ids=list(range(NCORES)))
    return assemble(res.results)


if __name__ == "__main__":
    rng = np.random.default_rng(0)
    inputs = {
        "query": rng.standard_normal((B, S, H), dtype=np.float32),
        "attention_mask": np.zeros((B, 1, 1, S), np.float32),
        "Wq": rng.standard_normal((H, H), dtype=np.float32) * 0.02,
        "bq": np.zeros(H, np.float32),
        "Wk": rng.standard_normal((H, H), dtype=np.float32) * 0.02,
        "bk": np.zeros(H, np.float32),
        "Wv": rng.standard_normal((H, H), dtype=np.float32) * 0.02,
        "bv": np.zeros(H, np.float32),
        "Wd": rng.standard_normal((H, H), dtype=np.float32) * 0.02,
        "bd": np.zeros(H, np.float32),
        "ln_gamma": np.ones(H, np.float32),
        "ln_beta": np.zeros(H, np.float32),
    }
    out = kernel(**inputs)
    print(out.shape, out.dtype)
